# revision 10
# baseline (speedup 1.0000x reference)
"""Channel-attention (CAM) kernel for Trainium2, 8 NeuronCores.

Reference computation (per batch b):
    A   = x[b].reshape(L, C)            # L = 48^3 = 110592, C = 256
    G   = A^T A                          # [C, C] Gram matrix
    S   = softmax(G, axis=-1)
    out = gamma * (A @ S) + x[b]

Sharding: L-parallel across the 8 cores (each core owns L/8 rows of both
batches).  Each core computes a partial Gram over its shard; a per-batch
bf16 AllReduce completes the [C, C] Grams; every core redundantly
computes softmax (tiny), scales it by gamma (zero gamma makes the second
matmul exactly zero, so the output is bit-exact in x), then computes its
shard of A @ (gamma*S) + x with a fp32 re-read of x for the residual.

Layout: [128, RPP, C] "supertiles" — partition p holds RPP consecutive
DRAM rows, giving RPP-KB DMA packets.  The Gram contraction is invariant
to the row permutation, and the transpose / second matmul / residual add
/ store all use the same permuted order consistently.

Phase 1 also produces the A^T tiles the second matmul needs (PE
identity-matmul transposes sharing the loaded bf16 tiles; both C-halves
of a subtile share one PSUM tile so a single scalar-engine copy drains
them) and keeps them resident in SBUF (14.2 MB).

Engine-queue discipline (FIFO queues make emission order matter):
  sync    = all x loads + Gram staging DMA
  scalar  = A^T copies, Gram staging copies, softmax (incl. its DMA)
  vector  = bf16 converts, softmax vector ops, residual adds + stores
  gpsimd  = gamma broadcast + the two AllReduces
softmax(b1) is emitted a few supertiles into phase 2 so its AllReduce
wait doesn't head-of-line-block the phase-2 adds on the vector queue.
"""

import numpy as np
from contextlib import ExitStack

import concourse.bass as bass
import concourse.tile as tile
from concourse import bacc, mybir
from concourse.bass import ts
from concourse.bass_utils import run_bass_kernel_spmd
from concourse.masks import make_identity

F32 = mybir.dt.float32
BF16 = mybir.dt.bfloat16
AF = mybir.ActivationFunctionType

N_CORES = 8
B = 2
L = 48 * 48 * 48          # 110592
C = 256
L_SH = L // N_CORES       # 13824 rows per core per batch
ROWS = B * L_SH           # 27648 rows per core
P = 128
RPP = 4                   # rows per partition per supertile
SROWS = P * RPP           # 512 rows per supertile
SPB = L_SH // SROWS       # 27 supertiles per batch
S_TOT = B * SPB           # 54 supertiles per core
NSM1 = 12                 # supertiles of phase 2 before softmax(b1) emission

_CACHE: dict = {}


def _build():
    nc = bacc.Bacc(
        "TRN2", target_bir_lowering=False, debug=False, num_devices=N_CORES
    )
    x_dram = nc.dram_tensor("x", [ROWS, C], F32, kind="ExternalInput")
    g_dram = nc.dram_tensor("gamma", [1, 1], F32, kind="ExternalInput")
    o_dram = nc.dram_tensor("out", [ROWS, C], F32, kind="ExternalOutput")
    cc_in = [
        nc.dram_tensor(f"cc_in{b}", [2 * P, C], BF16, kind="Internal")
        for b in range(B)
    ]
    cc_out = [
        nc.dram_tensor(f"cc_out{b}", [2 * P, C], BF16, kind="Internal")
        for b in range(B)
    ]
    X, GAM, OUT = x_dram.ap(), g_dram.ap(), o_dram.ap()

    def x_super(s):
        return X[ts(s, SROWS), :].rearrange("(p j) c -> p j c", j=RPP)

    def o_super(s):
        return OUT[ts(s, SROWS), :].rearrange("(p j) c -> p j c", j=RPP)

    with tile.TileContext(nc) as tc, ExitStack() as octx:
        constp = octx.enter_context(tc.tile_pool(name="const", bufs=1))
        ident = constp.tile([P, P], BF16, name="ident", tag="ident")
        make_identity(nc, ident[:])
        gam_sb = constp.tile([1, 1], F32, name="gam_sb", tag="gam_sb")
        nc.sync.dma_start(gam_sb[:], GAM[:, :])
        gam_bc = constp.tile([P, 1], F32, name="gam_bc", tag="gam_bc")
        nc.gpsimd.partition_broadcast(gam_bc[:], gam_sb[:])
        s_bf = [
            constp.tile([P, C], BF16, name=f"sbf{i}", tag=f"sbf{i}")
            for i in range(4)
        ]
        # A^T tiles, resident across both phases: one [128, 2, 128] bf16
        # tile per subtile (both C-halves of one transposed 128x256 block).
        atp = octx.enter_context(tc.tile_pool(name="atp", bufs=B * SPB * RPP))
        ats: dict = {}

        p1 = octx.enter_context(ExitStack())
        xp = p1.enter_context(tc.tile_pool(name="p1x", bufs=4))
        bp = p1.enter_context(tc.tile_pool(name="p1b", bufs=3))
        gp = p1.enter_context(tc.tile_pool(name="p1g", bufs=4))
        psg = p1.enter_context(tc.tile_pool(name="psg", bufs=1, space="PSUM"))
        pst = p1.enter_context(tc.tile_pool(name="pst", bufs=4, space="PSUM"))
        g_ps = [
            psg.tile([P, C], F32, name=f"gps{i}", tag=f"gps{i}")
            for i in range(4)
        ]

        # ---- phase 1: partial Gram G = A^T A and resident A^T tiles ----
        def phase1_batch(b):
            for si in range(SPB):
                s = b * SPB + si
                xt = xp.tile([P, RPP, C], F32, name="x1", tag="x1")
                nc.sync.dma_start(xt[:], x_super(s))
                xb = bp.tile([P, RPP, C], BF16, name="xb1", tag="xb1")
                nc.vector.tensor_copy(xb[:], xt[:])
                for j in range(RPP):
                    first = si == 0 and j == 0
                    last = si == SPB - 1 and j == RPP - 1
                    tp = pst.tile([P, 2, P], F32, name="tp", tag="tp")
                    nc.tensor.matmul(
                        g_ps[2 * b][:], xb[:, j, 0:P], xb[:, j, :],
                        start=first, stop=last,
                    )
                    nc.tensor.matmul(
                        tp[:, 0, :], xb[:, j, 0:P], ident[:],
                        start=True, stop=False,
                    )
                    nc.tensor.matmul(
                        g_ps[2 * b + 1][:], xb[:, j, P:C], xb[:, j, :],
                        start=first, stop=last,
                    )
                    nc.tensor.matmul(
                        tp[:, 1, :], xb[:, j, P:C], ident[:],
                        start=False, stop=True,
                    )
                    at = atp.tile([P, 2, P], BF16, name="at", tag="at")
                    nc.scalar.activation(at[:], tp[:], AF.Copy)
                    ats[(s, j)] = at

        def stage_and_ar(b):
            for m in range(2):
                gsb = gp.tile([P, C], BF16, name="gsb", tag="gsb")
                nc.vector.tensor_copy(gsb[:], g_ps[2 * b + m][:])
                nc.sync.dma_start(cc_in[b].ap()[ts(m, P), :], gsb[:])
            nc.gpsimd.collective_compute(
                "AllReduce",
                mybir.AluOpType.add,
                replica_groups=[list(range(N_CORES))],
                ins=[cc_in[b].ap()[:, :]],
                outs=[cc_out[b].ap()[:, :]],
            )

        # ---- softmax rows + fold gamma:  s_bf = gamma * softmax(G) ----
        def softmax(b, sp):
            for m in range(2):
                i = 2 * b + m
                gf = sp.tile([P, C], BF16, name="gf", tag="gf")
                nc.scalar.dma_start(gf[:], cc_out[b].ap()[ts(m, P), :])
                nmx = sp.tile([P, 1], F32, name="nmx", tag="nmx")
                nc.vector.tensor_reduce(
                    nmx[:],
                    gf[:],
                    axis=mybir.AxisListType.X,
                    op=mybir.AluOpType.max,
                    negate=True,
                )
                ex = sp.tile([P, C], F32, name="ex", tag="ex")
                ssum = sp.tile([P, 1], F32, name="ssum", tag="ssum")
                nc.scalar.activation(
                    ex[:], gf[:], AF.Exp, bias=nmx[:], scale=1.0, accum_out=ssum[:]
                )
                inv = sp.tile([P, 1], F32, name="inv", tag="inv")
                nc.vector.reciprocal(inv[:], ssum[:])
                sc = sp.tile([P, 1], F32, name="sc", tag="sc")
                nc.vector.tensor_mul(sc[:], inv[:], gam_bc[:])
                nc.scalar.activation(s_bf[i][:], ex[:], AF.Copy, scale=sc[:])

        phase1_batch(0)
        stage_and_ar(0)
        phase1_batch(1)
        stage_and_ar(1)
        p1.close()

        sm = octx.enter_context(ExitStack())
        sp = sm.enter_context(tc.tile_pool(name="smx", bufs=2))
        softmax(0, sp)

        # ---- phase 2: out = A @ s_bf + x ----
        with ExitStack() as p2:
            xp2 = p2.enter_context(tc.tile_pool(name="p2x", bufs=14))
            op2 = p2.enter_context(tc.tile_pool(name="p2o", bufs=3))
            psy = p2.enter_context(tc.tile_pool(name="psy", bufs=3, space="PSUM"))

            def mainwork(s):
                b = s // SPB
                xt = xp2.tile([P, RPP, C], F32, name="x2", tag="x2")
                nc.sync.dma_start(xt[:], x_super(s))
                y = psy.tile([P, RPP, C], F32, name="y", tag="y")
                for j in range(RPP):
                    a = ats.pop((s, j))
                    nc.tensor.matmul(
                        y[:, j, :], a[:, 0, :], s_bf[2 * b][:],
                        start=True, stop=False,
                    )
                    nc.tensor.matmul(
                        y[:, j, :], a[:, 1, :], s_bf[2 * b + 1][:],
                        start=False, stop=True,
                    )
                ot = op2.tile([P, RPP, C], F32, name="ot", tag="ot")
                nc.vector.tensor_add(ot[:], y[:], xt[:])
                nc.gpsimd.dma_start(o_super(s), ot[:])

            for s in range(NSM1):
                mainwork(s)
            softmax(1, sp)
            for s in range(NSM1, S_TOT):
                mainwork(s)
        sm.close()

    nc.compile()
    return nc


def _get_nc():
    if "nc" not in _CACHE:
        _CACHE["nc"] = _build()
    return _CACHE["nc"]


def kernel(x: np.ndarray, gamma: np.ndarray, **_kw) -> np.ndarray:
    nc = _get_nc()
    x = np.asarray(x, dtype=np.float32)
    orig_shape = x.shape
    x3 = x.reshape(B, L, C)
    gam = np.asarray(gamma, dtype=np.float32).reshape(1, 1)
    in_maps = []
    for k in range(N_CORES):
        shard = np.ascontiguousarray(
            x3[:, k * L_SH : (k + 1) * L_SH, :]
        ).reshape(ROWS, C)
        in_maps.append({"x": shard, "gamma": gam})
    res = run_bass_kernel_spmd(nc, in_maps, core_ids=list(range(N_CORES)))
    out = np.empty((B, L, C), dtype=np.float32)
    for k in range(N_CORES):
        out[:, k * L_SH : (k + 1) * L_SH, :] = res.results[k]["out"].reshape(
            B, L_SH, C
        )
    return out.reshape(orig_shape)
